# revision 2
# baseline (speedup 1.0000x reference)
"""Trainium2 Bass kernel for AlignmentContrastiveLoss (8-core SPMD).

Math: with conserved c_i = (cat_i < 3), key k_i = label_i + 512*graph_i
(non-conserved rows dropped),

  pos_cnt    = 1/2 (sum_L n_L^2 - sum_k n_k^2)
  sum_valid_sims = 1/2 (||U||_F^2 - ||W||_F^2)
      U[L,:] = sum_{i: l_i=L, c_i} e_i   (e = row-normalized embeddings)
      W[k,:] = sum_{i: k_i=k, c_i} e_i
  pos_sum    = pos_cnt - sum_valid_sims

Sharding: conserved rows are bucketed BY GRAPH on the host -- core c owns
graphs {2c, 2c+1}, i.e. exactly its 1024-key range, so W for those keys
is computed from only the core's own rows (8 single-tile one-hot matmuls,
one per 128-key block).  Normalization is folded into the one-hot LHS
(scaled by 1/||r||).  Negative pairs are sharded 625/core with their raw
rows host-gathered (pure indexing); sims are computed as <r1,r2>ยท
rsqrt(||r1||^2 ||r2||^2) on device.  One bf16 AllReduce [514,512] carries
U partials, per-label counts and the scalar partial sums.
"""

import os
import sys

import numpy as np

if "/opt/trn_rl_repo" not in sys.path:
    sys.path.insert(0, "/opt/trn_rl_repo")

# persistent jax/neuron compile cache: repeat invocations skip the NEFF build
os.environ.setdefault("JAX_COMPILATION_CACHE_DIR", "/tmp/jaxcache")
os.environ.setdefault("JAX_PERSISTENT_CACHE_MIN_COMPILE_TIME_SECS", "1")
os.environ.setdefault("JAX_PERSISTENT_CACHE_MIN_ENTRY_SIZE_BYTES", "0")

import concourse.mybir as mybir  # noqa: E402
import concourse.tile as tile  # noqa: E402
from concourse import bacc  # noqa: E402
from concourse.bass_utils import run_bass_kernel_spmd  # noqa: E402

# Problem constants (hardcoded per the self-contained-kernel contract).
N, D, S = 8192, 512, 5000
M = 8                 # cores
NB = 8                # key blocks per core (128 keys each; 1024 keys/core)
OWN = NB * 128        # own-row slots per core (128 per key block)
SP = S // M           # 625 pairs per core
NPT = 5               # neg pair tiles: 5 * 128 = 640 >= 625

F32 = mybir.dt.float32
BF16 = mybir.dt.bfloat16
I16 = mybir.dt.int16
ALU = mybir.AluOpType
ACTF = mybir.ActivationFunctionType
AX = mybir.AxisListType

_PROGRAM_CACHE = {}


def build_program():
    """Build + compile the (single) SPMD Bass program. Returns nc."""
    if "nc" in _PROGRAM_CACHE:
        return _PROGRAM_CACHE["nc"]

    nc = bacc.Bacc("TRN2", target_bir_lowering=False, debug=False, num_devices=M)

    own_d = nc.dram_tensor("own", [OWN, D], BF16, kind="ExternalInput")
    krel_d = nc.dram_tensor("krel", [128, NB], F32, kind="ExternalInput")
    nr1_d = nc.dram_tensor("nr1", [128, NPT, D], BF16, kind="ExternalInput")
    nr2_d = nc.dram_tensor("nr2", [128, NPT, D], BF16, kind="ExternalInput")
    meta_d = nc.dram_tensor("meta", [128, 32], F32, kind="ExternalInput")
    out_d = nc.dram_tensor("out", [1, 1], F32, kind="ExternalOutput")

    groups = [list(range(M))]

    with tile.TileContext(nc) as tc:
        with (
            tc.tile_pool(name="cst", bufs=1) as cst,
            tc.tile_pool(name="sb", bufs=2) as sb,
            tc.tile_pool(name="drp", bufs=1, space="DRAM") as drp,
        ):
            # ---- constants / metadata ----
            iota_t = cst.tile([128, 128], I16, name="iota_t")
            nc.gpsimd.iota(iota_t[:], pattern=[[1, 128]], base=0, channel_multiplier=0)
            ones_bf = cst.tile([128, 1], BF16, name="ones_bf")
            nc.vector.memset(ones_bf[:], 1.0)
            ones_f32 = cst.tile([128, 1], F32, name="ones_f32")
            nc.vector.memset(ones_f32[:], 1.0)

            krel = cst.tile([128, NB], F32, name="krel")
            nc.sync.dma_start(krel[:], krel_d[:, :])

            # neg-pair inputs (loaded early; consumed mid-kernel)
            g1 = cst.tile([128, NPT, D], BF16, name="g1")
            g2 = cst.tile([128, NPT, D], BF16, name="g2")
            meta = cst.tile([128, 32], F32, name="meta")
            nc.sync.dma_start(g1[:], nr1_d[:, :, :])
            nc.sync.dma_start(g2[:], nr2_d[:, :, :])
            nc.sync.dma_start(meta[:], meta_d[:, :])

            # ---- phase A: per key-block one-hot matmul ----
            # o_sum[:, 128b:128b+128] = one-hot of block b (1 if iota==krel col)
            o_sum = cst.tile([128, NB * 128], BF16, name="o_sum")
            pspW_cm = tc.tile_pool(name="pspW", bufs=1, space="PSUM")
            pspW = pspW_cm.__enter__()
            pw = [
                pspW.tile([128, D], F32, name=f"pw{b}", tag=f"pw{b}")
                for b in range(NB)
            ]
            for j in range(NB):
                e_t = sb.tile([128, D], BF16, name=f"e_{j}", tag="eload", bufs=4)
                nc.sync.dma_start(e_t[:], own_d[j * 128 : (j + 1) * 128, :])
                sqs = sb.tile([128, 1], F32, name=f"sqs_{j}", tag="sqs", bufs=4)
                scr = sb.tile([128, D], F32, name=f"scr_{j}", tag="scr", bufs=4)
                nc.scalar.activation(scr[:], e_t[:], ACTF.Square, accum_out=sqs[:])
                # inv = rsqrt(max(ss, eps)); pad rows are all-zero -> inv finite
                sqc = sb.tile([128, 1], F32, name=f"sqc_{j}", tag="sqc", bufs=4)
                nc.vector.tensor_scalar(sqc[:], sqs[:], 1e-12, None, ALU.max)
                inv = sb.tile([128, 1], F32, name=f"inv_{j}", tag="inv", bufs=4)
                nc.scalar.activation(inv[:], sqc[:], ACTF.Rsqrt)

                nc.vector.tensor_scalar(
                    o_sum[:, j * 128 : (j + 1) * 128],
                    iota_t[:],
                    krel[:, j : j + 1],
                    None,
                    ALU.is_equal,
                )
                soh = sb.tile([128, 128], BF16, name=f"soh_{j}", tag="soh", bufs=4)
                nc.vector.tensor_scalar(
                    soh[:], o_sum[:, j * 128 : (j + 1) * 128], inv[:], None, ALU.mult
                )
                nc.tensor.matmul(pw[j][:, :], soh[:], e_t[:], start=True, stop=True)

            # ---- phase B: evacuate W -> U partial (bf16) + ||W||^2 ----
            u_sb = []
            for b in range(4):
                # HW: tensor_tensor may read only one PSUM operand - stage one
                wcp = sb.tile([128, D], F32, name=f"wcp_{b}", tag="wcp")
                nc.scalar.activation(wcp[:], pw[b][:, :], ACTF.Copy)
                u_t = sb.tile([128, D], BF16, name=f"u_{b}", tag=f"usb{b}")
                nc.vector.tensor_tensor(u_t[:], wcp[:], pw[b + 4][:, :], ALU.add)
                u_sb.append(u_t)

            wsqc = sb.tile([128, 8], F32, name="wsqc")
            for m in range(NB):
                wscr = sb.tile([128, D], F32, name=f"wscr_{m}", tag="wscr")
                nc.scalar.activation(
                    wscr[:], pw[m][:, :], ACTF.Square, accum_out=wsqc[:, m : m + 1]
                )

            pspW_cm.__exit__(None, None, None)

            # ---- phase C: key counts n_k = ones^T @ o_sum ----
            psp2_cm = tc.tile_pool(name="psp2", bufs=1, space="PSUM")
            psp2 = psp2_cm.__enter__()
            psc = psp2.tile([1, 1024], F32, name="psc")
            nc.tensor.matmul(
                psc[0:1, 0:512], ones_bf[:], o_sum[:, 0:512], start=True, stop=True
            )
            nc.tensor.matmul(
                psc[0:1, 512:1024], ones_bf[:], o_sum[:, 512:1024],
                start=True, stop=True,
            )
            # per-label counts partial (sum this core's two graphs), bf16 exact
            nlc = sb.tile([1, 512], F32, name="nlc")
            nc.scalar.activation(nlc[:], psc[0:1, 0:512], ACTF.Copy)
            nl_sb = sb.tile([1, 512], BF16, name="nl_sb")
            nc.vector.tensor_tensor(nl_sb[:], nlc[:], psc[0:1, 512:1024], ALU.add)
            # sum_k n_k^2 (local scalar)
            scscr = sb.tile([1, 1024], F32, name="scscr")
            snk2 = sb.tile([1, 1], F32, name="snk2")
            nc.scalar.activation(scscr[:], psc[0:1, :], ACTF.Square, accum_out=snk2[:])

            # ---- phase D: negative pairs ----
            sq1 = sb.tile([128, NPT, D], BF16, name="sq1")
            nc.vector.tensor_tensor(sq1[:], g1[:], g1[:], ALU.mult)
            ss1 = sb.tile([128, NPT], F32, name="ss1")
            nc.vector.tensor_reduce(ss1[:], sq1[:], axis=AX.X, op=ALU.add)
            sq2 = sb.tile([128, NPT, D], BF16, name="sq2")
            nc.vector.tensor_tensor(sq2[:], g2[:], g2[:], ALU.mult)
            ss2 = sb.tile([128, NPT], F32, name="ss2")
            nc.vector.tensor_reduce(ss2[:], sq2[:], axis=AX.X, op=ALU.add)
            prod = sb.tile([128, NPT, D], BF16, name="prod")
            nc.vector.tensor_tensor(prod[:], g1[:], g2[:], ALU.mult)
            dots = sb.tile([128, NPT], F32, name="dots")
            nc.vector.tensor_reduce(dots[:], prod[:], axis=AX.X, op=ALU.add)

            ssp = sb.tile([128, NPT], F32, name="ssp")
            nc.vector.tensor_tensor(ssp[:], ss1[:], ss2[:], ALU.mult)
            nc.vector.tensor_scalar(ssp[:], ssp[:], 1e-12, None, ALU.max)
            inv12 = sb.tile([128, NPT], F32, name="inv12")
            nc.scalar.activation(inv12[:], ssp[:], ACTF.Rsqrt)
            sim = sb.tile([128, NPT], F32, name="sim")
            nc.vector.tensor_tensor(sim[:], dots[:], inv12[:], ALU.mult)
            pen = sb.tile([128, NPT], F32, name="pen")
            nc.vector.tensor_scalar(pen[:], sim[:], 0.0, None, ALU.max)

            # masks: (l1 != l2) & (g1 != g2) & (cons1 | cons2)
            vmask = sb.tile([128, NPT], F32, name="vmask")
            nc.vector.tensor_tensor(
                vmask[:], meta[:, 0:NPT], meta[:, NPT : 2 * NPT], ALU.not_equal
            )
            gmask = sb.tile([128, NPT], F32, name="gmask")
            nc.vector.tensor_tensor(
                gmask[:], meta[:, 2 * NPT : 3 * NPT], meta[:, 3 * NPT : 4 * NPT],
                ALU.not_equal,
            )
            nc.vector.tensor_tensor(vmask[:], vmask[:], gmask[:], ALU.mult)
            c1c = sb.tile([128, NPT], F32, name="c1c")
            nc.vector.tensor_scalar(
                c1c[:], meta[:, 4 * NPT : 5 * NPT], 2.5, None, ALU.is_lt
            )
            c2c = sb.tile([128, NPT], F32, name="c2c")
            nc.vector.tensor_scalar(
                c2c[:], meta[:, 5 * NPT : 6 * NPT], 2.5, None, ALU.is_lt
            )
            nc.vector.tensor_tensor(c1c[:], c1c[:], c2c[:], ALU.add)
            cmask = sb.tile([128, NPT], F32, name="cmask")
            nc.vector.tensor_scalar(cmask[:], c1c[:], 0.5, None, ALU.is_gt)
            nc.vector.tensor_tensor(vmask[:], vmask[:], cmask[:], ALU.mult)
            nc.vector.tensor_tensor(pen[:], pen[:], vmask[:], ALU.mult)

            # cols: 0 = ||W||^2 partial, 1 = neg_sum, 2 = neg_cnt
            cols3 = sb.tile([128, 4], F32, name="cols3")
            nc.vector.memset(cols3[:], 0.0)
            nc.vector.tensor_reduce(cols3[:, 0:1], wsqc[:], axis=AX.X, op=ALU.add)
            nc.vector.tensor_reduce(cols3[:, 1:2], pen[:], axis=AX.X, op=ALU.add)
            nc.vector.tensor_reduce(cols3[:, 2:3], vmask[:], axis=AX.X, op=ALU.add)
            psum_s = psp2.tile([1, 8], F32, name="psum_s")
            nc.tensor.matmul(
                psum_s[0:1, 0:4], ones_f32[:], cols3[:], start=True, stop=True
            )
            sc_row = sb.tile([1, 512], BF16, name="sc_row")
            nc.vector.memset(sc_row[:], 0.0)
            nc.vector.tensor_copy(sc_row[:, 0:3], psum_s[0:1, 0:3])
            nc.vector.tensor_copy(sc_row[:, 3:4], snk2[:])

            # ---- phase E: pack partials, all-reduce (bf16) ----
            arb = drp.tile([514, 512], BF16, name="arb")
            for b in range(4):
                nc.sync.dma_start(arb[b * 128 : (b + 1) * 128, :], u_sb[b][:])
            nc.sync.dma_start(arb[512:513, :], nl_sb[:])
            nc.sync.dma_start(arb[513:514, :], sc_row[:])
            aro = drp.tile([514, 512], BF16, name="aro", addr_space="Shared")
            nc.gpsimd.collective_compute(
                "AllReduce",
                ALU.add,
                replica_groups=groups,
                ins=[arb.opt()],
                outs=[aro.opt()],
            )

            # ---- phase F: final scalar ----
            uf = sb.tile([128, 4, 512], BF16, name="uf")
            nc.sync.dma_start(
                uf[:], aro[0:512, :].rearrange("(b p) d -> p b d", p=128)
            )
            uscr = sb.tile([128, 4, 512], F32, name="uscr")
            u2red = sb.tile([128, 1], F32, name="u2red")
            nc.scalar.activation(uscr[:], uf[:], ACTF.Square, accum_out=u2red[:])
            psum_u = psp2.tile([1, 8], F32, name="psum_u")
            nc.tensor.matmul(
                psum_u[0:1, 0:1], ones_f32[:], u2red[:], start=True, stop=True
            )

            nlf = sb.tile([1, 512], BF16, name="nlf")
            nc.sync.dma_start(nlf[:], aro[512:513, :])
            nlscr = sb.tile([1, 512], F32, name="nlscr")
            nl2 = sb.tile([1, 1], F32, name="nl2")
            nc.scalar.activation(nlscr[:], nlf[:], ACTF.Square, accum_out=nl2[:])
            scfb = sb.tile([1, 512], BF16, name="scfb")
            nc.sync.dma_start(scfb[:], aro[513:514, :])
            scf = sb.tile([1, 512], F32, name="scf")
            nc.vector.tensor_copy(scf[:], scfb[:])

            # pos_cnt = 0.5*(sum nL^2 - sum nk^2)
            pc = sb.tile([1, 1], F32, name="pc")
            nc.vector.tensor_tensor(pc[:], nl2[:], scf[:, 3:4], ALU.subtract)
            nc.vector.tensor_scalar(pc[:], pc[:], 0.5, None, ALU.mult)
            # pos_sumsim = 0.5*(||U||^2 - ||W||^2)
            ps_ = sb.tile([1, 1], F32, name="ps_")
            nc.vector.tensor_tensor(
                ps_[:], psum_u[0:1, 0:1], scf[:, 0:1], ALU.subtract
            )
            nc.vector.tensor_scalar(ps_[:], ps_[:], 0.5, None, ALU.mult)
            # pos_loss = (pos_cnt - pos_sumsim) / max(pos_cnt,1) * (pos_cnt>0)
            psum_t = sb.tile([1, 1], F32, name="psum_t")
            nc.vector.tensor_tensor(psum_t[:], pc[:], ps_[:], ALU.subtract)
            den = sb.tile([1, 1], F32, name="den")
            nc.vector.tensor_scalar(den[:], pc[:], 1.0, None, ALU.max)
            rec = sb.tile([1, 1], F32, name="rec")
            nc.vector.reciprocal(rec[:], den[:])
            msk = sb.tile([1, 1], F32, name="msk")
            nc.vector.tensor_scalar(msk[:], pc[:], 0.0, None, ALU.is_gt)
            ploss = sb.tile([1, 1], F32, name="ploss")
            nc.vector.scalar_tensor_tensor(
                ploss[:], psum_t[:], rec[:], msk[:], ALU.mult, ALU.mult
            )
            # neg_loss
            den2 = sb.tile([1, 1], F32, name="den2")
            nc.vector.tensor_scalar(den2[:], scf[:, 2:3], 1.0, None, ALU.max)
            rec2 = sb.tile([1, 1], F32, name="rec2")
            nc.vector.reciprocal(rec2[:], den2[:])
            msk2 = sb.tile([1, 1], F32, name="msk2")
            nc.vector.tensor_scalar(msk2[:], scf[:, 2:3], 0.0, None, ALU.is_gt)
            nloss = sb.tile([1, 1], F32, name="nloss")
            nc.vector.scalar_tensor_tensor(
                nloss[:], scf[:, 1:2], rec2[:], msk2[:], ALU.mult, ALU.mult
            )

            outv = sb.tile([1, 1], F32, name="outv")
            nc.vector.tensor_tensor(outv[:], ploss[:], nloss[:], ALU.add)
            nc.sync.dma_start(out_d[:, :], outv[:])
            psp2_cm.__exit__(None, None, None)

    nc.compile()
    _PROGRAM_CACHE["nc"] = nc
    return nc


def make_in_maps(embeddings, labels, graph_ids, categories, idx1, idx2):
    """Host-side sharding / layout marshaling. Returns per-core input dicts."""
    import ml_dtypes

    emb = np.ascontiguousarray(
        np.asarray(embeddings, dtype=np.float32).astype(ml_dtypes.bfloat16)
    )
    l = np.asarray(labels).astype(np.int64)
    g = np.asarray(graph_ids).astype(np.int64)
    c = np.asarray(categories).astype(np.int64)
    i1 = np.asarray(idx1).astype(np.int64)
    i2 = np.asarray(idx2).astype(np.int64)
    assert emb.shape == (N, D) and l.shape == (N,) and i1.shape == (S,)

    cons = c < 3

    in_maps = []
    for core in range(M):
        own = np.zeros((OWN, D), dtype=ml_dtypes.bfloat16)
        krel = np.full((128, NB), 999.0, dtype=np.float32)
        for b in range(NB):
            gb = 2 * core + b // 4
            lo = 128 * (b % 4)
            sel = np.nonzero(cons & (g == gb) & (l >= lo) & (l < lo + 128))[0]
            nb_ = len(sel)
            assert nb_ <= 128, f"key-block overflow: {nb_} rows"
            own[b * 128 : b * 128 + nb_] = emb[sel]
            krel[:nb_, b] = (l[sel] - lo).astype(np.float32)

        # negative pairs: q-th pair of this core at [q % 128, q // 128]
        sl = slice(core * SP, (core + 1) * SP)
        p1 = np.zeros(NPT * 128, np.int64)
        p2 = np.zeros(NPT * 128, np.int64)
        p1[:SP] = i1[sl]
        p2[:SP] = i2[sl]
        nr1 = np.ascontiguousarray(
            emb[p1].reshape(NPT, 128, D).transpose(1, 0, 2)
        )
        nr2 = np.ascontiguousarray(
            emb[p2].reshape(NPT, 128, D).transpose(1, 0, 2)
        )
        meta = np.zeros((128, 32), dtype=np.float32)
        for f, arr in enumerate((l[p1], l[p2], g[p1], g[p2], c[p1], c[p2])):
            meta[:, f * NPT : (f + 1) * NPT] = arr.reshape(NPT, 128).T
        # pad pairs (q >= SP) share row 0 on both sides -> same graph -> masked,
        # but force-invalidate via equal labels in case row 0's graph differs
        padmask = np.zeros(NPT * 128, bool)
        padmask[SP:] = True
        pm2 = padmask.reshape(NPT, 128).T
        meta[:, 0:NPT][pm2] = 0.0
        meta[:, NPT : 2 * NPT][pm2] = 0.0

        in_maps.append(
            {
                "own": own,
                "krel": krel,
                "nr1": nr1,
                "nr2": nr2,
                "meta": meta,
            }
        )
    return in_maps


def kernel(embeddings, labels, graph_ids, categories, idx1, idx2):
    nc = build_program()
    in_maps = make_in_maps(embeddings, labels, graph_ids, categories, idx1, idx2)
    res = run_bass_kernel_spmd(nc, in_maps, list(range(M)))
    out = np.asarray(res.results[0]["out"], dtype=np.float32)
    return out.reshape(())


# revision 4
# speedup vs baseline: 3.2388x; 3.2388x over previous
"""Trainium2 Bass kernel for AlignmentContrastiveLoss (8-core SPMD).

Math: with conserved c_i = (cat_i < 3), key k_i = label_i + 512*graph_i
(non-conserved rows dropped),

  pos_cnt    = 1/2 (sum_L n_L^2 - sum_k n_k^2)
  sum_valid_sims = 1/2 (||U||_F^2 - ||W||_F^2)
      U[L,:] = sum_{i: l_i=L, c_i} e_i   (e = row-normalized embeddings)
      W[k,:] = sum_{i: k_i=k, c_i} e_i
  pos_sum    = pos_cnt - sum_valid_sims

Sharding: conserved rows are bucketed BY GRAPH on the host -- core c owns
graphs {2c, 2c+1}, i.e. exactly its 1024-key range, so W for those keys
is computed from only the core's own rows (8 single-tile one-hot matmuls,
one per 128-key block).  Normalization is folded into the one-hot LHS
(scaled by 1/||r||).  Negative pairs are sharded 625/core with their raw
rows host-gathered (pure indexing); sims are computed as <r1,r2>ยท
rsqrt(||r1||^2 ||r2||^2) on device.  One bf16 AllReduce [514,512] carries
U partials, per-label counts and the scalar partial sums.
"""

import os
import sys

import numpy as np

if "/opt/trn_rl_repo" not in sys.path:
    sys.path.insert(0, "/opt/trn_rl_repo")

# persistent jax/neuron compile cache: repeat invocations skip the NEFF build
os.environ.setdefault("JAX_COMPILATION_CACHE_DIR", "/tmp/jaxcache")
os.environ.setdefault("JAX_PERSISTENT_CACHE_MIN_COMPILE_TIME_SECS", "1")
os.environ.setdefault("JAX_PERSISTENT_CACHE_MIN_ENTRY_SIZE_BYTES", "0")

import concourse.mybir as mybir  # noqa: E402
import concourse.tile as tile  # noqa: E402
from concourse import bacc  # noqa: E402
from concourse.bass_utils import run_bass_kernel_spmd  # noqa: E402

# Problem constants (hardcoded per the self-contained-kernel contract).
N, D, S = 8192, 512, 5000
M = 8                 # cores
NB = 8                # key blocks per core (128 keys each; 1024 keys/core)
OWN = NB * 128        # own-row slots per core (128 per key block)
SP = S // M           # 625 pairs per core
NPT = 5               # neg pair tiles: 5 * 128 = 640 >= 625

F32 = mybir.dt.float32
BF16 = mybir.dt.bfloat16
I16 = mybir.dt.int16
ALU = mybir.AluOpType
ACTF = mybir.ActivationFunctionType
AX = mybir.AxisListType

_PROGRAM_CACHE = {}


def build_program():
    """Build + compile the (single) SPMD Bass program. Returns nc."""
    if "nc" in _PROGRAM_CACHE:
        return _PROGRAM_CACHE["nc"]

    nc = bacc.Bacc("TRN2", target_bir_lowering=False, debug=False, num_devices=M)

    own_d = nc.dram_tensor("own", [OWN, D], BF16, kind="ExternalInput")
    krel_d = nc.dram_tensor("krel", [128, NB], F32, kind="ExternalInput")
    nr1_d = nc.dram_tensor("nr1", [128, NPT, D], BF16, kind="ExternalInput")
    nr2_d = nc.dram_tensor("nr2", [128, NPT, D], BF16, kind="ExternalInput")
    meta_d = nc.dram_tensor("meta", [128, 32], F32, kind="ExternalInput")
    out_d = nc.dram_tensor("out", [1, 1], F32, kind="ExternalOutput")

    groups = [list(range(M))]

    with tile.TileContext(nc) as tc:
        with (
            tc.tile_pool(name="cst", bufs=1) as cst,
            tc.tile_pool(name="sb", bufs=2) as sb,
            tc.tile_pool(name="drp", bufs=1, space="DRAM") as drp,
        ):
            # ---- constants / metadata ----
            iota_t = cst.tile([128, 128], I16, name="iota_t")
            nc.gpsimd.iota(iota_t[:], pattern=[[1, 128]], base=0, channel_multiplier=0)
            ones_bf = cst.tile([128, 1], BF16, name="ones_bf")
            nc.vector.memset(ones_bf[:], 1.0)
            ones_f32 = cst.tile([128, 1], F32, name="ones_f32")
            nc.vector.memset(ones_f32[:], 1.0)

            krel = cst.tile([128, NB], F32, name="krel")
            nc.sync.dma_start(krel[:], krel_d[:, :])

            # neg-pair inputs (loaded early; consumed mid-kernel)
            g1 = cst.tile([128, NPT, D], BF16, name="g1")
            g2 = cst.tile([128, NPT, D], BF16, name="g2")
            meta = cst.tile([128, 32], F32, name="meta")
            nc.sync.dma_start(g1[:], nr1_d[:, :, :])
            nc.sync.dma_start(g2[:], nr2_d[:, :, :])
            nc.sync.dma_start(meta[:], meta_d[:, :])

            # ---- phase A: per key-block one-hot matmul ----
            # o_sum[:, 128b:128b+128] = one-hot of block b (1 if iota==krel col)
            o_sum = cst.tile([128, NB * 128], BF16, name="o_sum")
            pspW_cm = tc.tile_pool(name="pspW", bufs=1, space="PSUM")
            pspW = pspW_cm.__enter__()
            pw = [
                pspW.tile([128, D], F32, name=f"pw{b}", tag=f"pw{b}")
                for b in range(NB)
            ]
            for j in range(NB):
                e_t = sb.tile([128, D], BF16, name=f"e_{j}", tag="eload", bufs=4)
                nc.sync.dma_start(e_t[:], own_d[j * 128 : (j + 1) * 128, :])
                sqs = sb.tile([128, 1], F32, name=f"sqs_{j}", tag="sqs", bufs=4)
                scr = sb.tile([128, D], F32, name=f"scr_{j}", tag="scr", bufs=4)
                nc.scalar.activation(scr[:], e_t[:], ACTF.Square, accum_out=sqs[:])
                # inv = rsqrt(max(ss, eps)); pad rows are all-zero -> inv finite
                sqc = sb.tile([128, 1], F32, name=f"sqc_{j}", tag="sqc", bufs=4)
                nc.vector.tensor_scalar(sqc[:], sqs[:], 1e-12, None, ALU.max)
                nrm = sb.tile([128, 1], F32, name=f"nrm_{j}", tag="nrm", bufs=4)
                nc.scalar.activation(nrm[:], sqc[:], ACTF.Sqrt)
                inv = sb.tile([128, 1], F32, name=f"inv_{j}", tag="inv", bufs=4)
                nc.vector.reciprocal(inv[:], nrm[:])

                nc.vector.tensor_scalar(
                    o_sum[:, j * 128 : (j + 1) * 128],
                    iota_t[:],
                    krel[:, j : j + 1],
                    None,
                    ALU.is_equal,
                )
                soh = sb.tile([128, 128], BF16, name=f"soh_{j}", tag="soh", bufs=4)
                nc.vector.tensor_scalar(
                    soh[:], o_sum[:, j * 128 : (j + 1) * 128], inv[:], None, ALU.mult
                )
                nc.tensor.matmul(pw[j][:, :], soh[:], e_t[:], start=True, stop=True)

            # ---- phase B: evacuate W -> U partial (bf16) + ||W||^2 ----
            u_sb = []
            for b in range(4):
                # HW: tensor_tensor may read only one PSUM operand - stage one
                wcp = sb.tile([128, D], F32, name=f"wcp_{b}", tag="wcp")
                nc.scalar.activation(wcp[:], pw[b][:, :], ACTF.Copy)
                u_t = sb.tile([128, D], BF16, name=f"u_{b}", tag=f"usb{b}")
                nc.vector.tensor_tensor(u_t[:], wcp[:], pw[b + 4][:, :], ALU.add)
                u_sb.append(u_t)

            wsqc = sb.tile([128, 8], F32, name="wsqc")
            for m in range(NB):
                wscr = sb.tile([128, D], F32, name=f"wscr_{m}", tag="wscr")
                nc.scalar.activation(
                    wscr[:], pw[m][:, :], ACTF.Square, accum_out=wsqc[:, m : m + 1]
                )

            pspW_cm.__exit__(None, None, None)

            # ---- phase C: key counts n_k = ones^T @ o_sum ----
            psp2_cm = tc.tile_pool(name="psp2", bufs=1, space="PSUM")
            psp2 = psp2_cm.__enter__()
            psc = psp2.tile([1, 1024], F32, name="psc")
            nc.tensor.matmul(
                psc[0:1, 0:512], ones_bf[:], o_sum[:, 0:512], start=True, stop=True
            )
            nc.tensor.matmul(
                psc[0:1, 512:1024], ones_bf[:], o_sum[:, 512:1024],
                start=True, stop=True,
            )
            # per-label counts partial (sum this core's two graphs), bf16 exact
            nlc = sb.tile([1, 512], F32, name="nlc")
            nc.scalar.activation(nlc[:], psc[0:1, 0:512], ACTF.Copy)
            nl_sb = sb.tile([1, 512], BF16, name="nl_sb")
            nc.vector.tensor_tensor(nl_sb[:], nlc[:], psc[0:1, 512:1024], ALU.add)
            # sum_k n_k^2 (local scalar)
            scscr = sb.tile([1, 1024], F32, name="scscr")
            snk2 = sb.tile([1, 1], F32, name="snk2")
            nc.scalar.activation(scscr[:], psc[0:1, :], ACTF.Square, accum_out=snk2[:])

            # ---- phase D: negative pairs ----
            sq1 = sb.tile([128, NPT, D], BF16, name="sq1")
            nc.vector.tensor_tensor(sq1[:], g1[:], g1[:], ALU.mult)
            ss1 = sb.tile([128, NPT], F32, name="ss1")
            nc.vector.tensor_reduce(ss1[:], sq1[:], axis=AX.X, op=ALU.add)
            sq2 = sb.tile([128, NPT, D], BF16, name="sq2")
            nc.vector.tensor_tensor(sq2[:], g2[:], g2[:], ALU.mult)
            ss2 = sb.tile([128, NPT], F32, name="ss2")
            nc.vector.tensor_reduce(ss2[:], sq2[:], axis=AX.X, op=ALU.add)
            prod = sb.tile([128, NPT, D], BF16, name="prod")
            nc.vector.tensor_tensor(prod[:], g1[:], g2[:], ALU.mult)
            dots = sb.tile([128, NPT], F32, name="dots")
            nc.vector.tensor_reduce(dots[:], prod[:], axis=AX.X, op=ALU.add)

            ssp = sb.tile([128, NPT], F32, name="ssp")
            nc.vector.tensor_tensor(ssp[:], ss1[:], ss2[:], ALU.mult)
            nc.vector.tensor_scalar(ssp[:], ssp[:], 1e-12, None, ALU.max)
            s12 = sb.tile([128, NPT], F32, name="s12")
            nc.scalar.activation(s12[:], ssp[:], ACTF.Sqrt)
            inv12 = sb.tile([128, NPT], F32, name="inv12")
            nc.vector.reciprocal(inv12[:], s12[:])
            sim = sb.tile([128, NPT], F32, name="sim")
            nc.vector.tensor_tensor(sim[:], dots[:], inv12[:], ALU.mult)
            pen = sb.tile([128, NPT], F32, name="pen")
            nc.vector.tensor_scalar(pen[:], sim[:], 0.0, None, ALU.max)

            # masks: (l1 != l2) & (g1 != g2) & (cons1 | cons2)
            vmask = sb.tile([128, NPT], F32, name="vmask")
            nc.vector.tensor_tensor(
                vmask[:], meta[:, 0:NPT], meta[:, NPT : 2 * NPT], ALU.not_equal
            )
            gmask = sb.tile([128, NPT], F32, name="gmask")
            nc.vector.tensor_tensor(
                gmask[:], meta[:, 2 * NPT : 3 * NPT], meta[:, 3 * NPT : 4 * NPT],
                ALU.not_equal,
            )
            nc.vector.tensor_tensor(vmask[:], vmask[:], gmask[:], ALU.mult)
            c1c = sb.tile([128, NPT], F32, name="c1c")
            nc.vector.tensor_scalar(
                c1c[:], meta[:, 4 * NPT : 5 * NPT], 2.5, None, ALU.is_lt
            )
            c2c = sb.tile([128, NPT], F32, name="c2c")
            nc.vector.tensor_scalar(
                c2c[:], meta[:, 5 * NPT : 6 * NPT], 2.5, None, ALU.is_lt
            )
            nc.vector.tensor_tensor(c1c[:], c1c[:], c2c[:], ALU.add)
            cmask = sb.tile([128, NPT], F32, name="cmask")
            nc.vector.tensor_scalar(cmask[:], c1c[:], 0.5, None, ALU.is_gt)
            nc.vector.tensor_tensor(vmask[:], vmask[:], cmask[:], ALU.mult)
            nc.vector.tensor_tensor(pen[:], pen[:], vmask[:], ALU.mult)

            # cols: 0 = ||W||^2 partial, 1 = neg_sum, 2 = neg_cnt
            cols3 = sb.tile([128, 4], F32, name="cols3")
            nc.vector.memset(cols3[:], 0.0)
            nc.vector.tensor_reduce(cols3[:, 0:1], wsqc[:], axis=AX.X, op=ALU.add)
            nc.vector.tensor_reduce(cols3[:, 1:2], pen[:], axis=AX.X, op=ALU.add)
            nc.vector.tensor_reduce(cols3[:, 2:3], vmask[:], axis=AX.X, op=ALU.add)
            psum_s = psp2.tile([1, 8], F32, name="psum_s")
            nc.tensor.matmul(
                psum_s[0:1, 0:4], ones_f32[:], cols3[:], start=True, stop=True
            )
            sc_row = sb.tile([1, 512], BF16, name="sc_row")
            nc.vector.memset(sc_row[:], 0.0)
            nc.vector.tensor_copy(sc_row[:, 0:3], psum_s[0:1, 0:3])
            nc.vector.tensor_copy(sc_row[:, 3:4], snk2[:])

            # ---- phase E: pack partials, all-reduce (bf16) ----
            arb = drp.tile([514, 512], BF16, name="arb")
            for b in range(4):
                nc.sync.dma_start(arb[b * 128 : (b + 1) * 128, :], u_sb[b][:])
            nc.sync.dma_start(arb[512:513, :], nl_sb[:])
            nc.sync.dma_start(arb[513:514, :], sc_row[:])
            aro = drp.tile([514, 512], BF16, name="aro", addr_space="Shared")
            nc.gpsimd.collective_compute(
                "AllReduce",
                ALU.add,
                replica_groups=groups,
                ins=[arb.opt()],
                outs=[aro.opt()],
            )

            # ---- phase F: final scalar ----
            uf = sb.tile([128, 4, 512], BF16, name="uf")
            nc.sync.dma_start(
                uf[:], aro[0:512, :].rearrange("(b p) d -> p b d", p=128)
            )
            uscr = sb.tile([128, 4, 512], F32, name="uscr")
            u2red = sb.tile([128, 1], F32, name="u2red")
            nc.scalar.activation(uscr[:], uf[:], ACTF.Square, accum_out=u2red[:])
            psum_u = psp2.tile([1, 8], F32, name="psum_u")
            nc.tensor.matmul(
                psum_u[0:1, 0:1], ones_f32[:], u2red[:], start=True, stop=True
            )

            nlf = sb.tile([1, 512], BF16, name="nlf")
            nc.sync.dma_start(nlf[:], aro[512:513, :])
            nlscr = sb.tile([1, 512], F32, name="nlscr")
            nl2 = sb.tile([1, 1], F32, name="nl2")
            nc.scalar.activation(nlscr[:], nlf[:], ACTF.Square, accum_out=nl2[:])
            scfb = sb.tile([1, 512], BF16, name="scfb")
            nc.sync.dma_start(scfb[:], aro[513:514, :])
            scf = sb.tile([1, 512], F32, name="scf")
            nc.vector.tensor_copy(scf[:], scfb[:])

            # pos_cnt = 0.5*(sum nL^2 - sum nk^2)
            pc = sb.tile([1, 1], F32, name="pc")
            nc.vector.tensor_tensor(pc[:], nl2[:], scf[:, 3:4], ALU.subtract)
            nc.vector.tensor_scalar(pc[:], pc[:], 0.5, None, ALU.mult)
            # pos_sumsim = 0.5*(||U||^2 - ||W||^2)
            ps_ = sb.tile([1, 1], F32, name="ps_")
            nc.vector.tensor_tensor(
                ps_[:], psum_u[0:1, 0:1], scf[:, 0:1], ALU.subtract
            )
            nc.vector.tensor_scalar(ps_[:], ps_[:], 0.5, None, ALU.mult)
            # pos_loss = (pos_cnt - pos_sumsim) / max(pos_cnt,1) * (pos_cnt>0)
            psum_t = sb.tile([1, 1], F32, name="psum_t")
            nc.vector.tensor_tensor(psum_t[:], pc[:], ps_[:], ALU.subtract)
            den = sb.tile([1, 1], F32, name="den")
            nc.vector.tensor_scalar(den[:], pc[:], 1.0, None, ALU.max)
            rec = sb.tile([1, 1], F32, name="rec")
            nc.vector.reciprocal(rec[:], den[:])
            msk = sb.tile([1, 1], F32, name="msk")
            nc.vector.tensor_scalar(msk[:], pc[:], 0.0, None, ALU.is_gt)
            ploss = sb.tile([1, 1], F32, name="ploss")
            nc.vector.scalar_tensor_tensor(
                ploss[:], psum_t[:], rec[:], msk[:], ALU.mult, ALU.mult
            )
            # neg_loss
            den2 = sb.tile([1, 1], F32, name="den2")
            nc.vector.tensor_scalar(den2[:], scf[:, 2:3], 1.0, None, ALU.max)
            rec2 = sb.tile([1, 1], F32, name="rec2")
            nc.vector.reciprocal(rec2[:], den2[:])
            msk2 = sb.tile([1, 1], F32, name="msk2")
            nc.vector.tensor_scalar(msk2[:], scf[:, 2:3], 0.0, None, ALU.is_gt)
            nloss = sb.tile([1, 1], F32, name="nloss")
            nc.vector.scalar_tensor_tensor(
                nloss[:], scf[:, 1:2], rec2[:], msk2[:], ALU.mult, ALU.mult
            )

            outv = sb.tile([1, 1], F32, name="outv")
            nc.vector.tensor_tensor(outv[:], ploss[:], nloss[:], ALU.add)
            nc.sync.dma_start(out_d[:, :], outv[:])
            psp2_cm.__exit__(None, None, None)

    nc.compile()
    _PROGRAM_CACHE["nc"] = nc
    return nc


def make_in_maps(embeddings, labels, graph_ids, categories, idx1, idx2):
    """Host-side sharding / layout marshaling. Returns per-core input dicts."""
    import ml_dtypes

    emb = np.ascontiguousarray(
        np.asarray(embeddings, dtype=np.float32).astype(ml_dtypes.bfloat16)
    )
    l = np.asarray(labels).astype(np.int64)
    g = np.asarray(graph_ids).astype(np.int64)
    c = np.asarray(categories).astype(np.int64)
    i1 = np.asarray(idx1).astype(np.int64)
    i2 = np.asarray(idx2).astype(np.int64)
    assert emb.shape == (N, D) and l.shape == (N,) and i1.shape == (S,)

    cons = c < 3

    in_maps = []
    for core in range(M):
        own = np.zeros((OWN, D), dtype=ml_dtypes.bfloat16)
        krel = np.full((128, NB), 999.0, dtype=np.float32)
        for b in range(NB):
            gb = 2 * core + b // 4
            lo = 128 * (b % 4)
            sel = np.nonzero(cons & (g == gb) & (l >= lo) & (l < lo + 128))[0]
            nb_ = len(sel)
            assert nb_ <= 128, f"key-block overflow: {nb_} rows"
            own[b * 128 : b * 128 + nb_] = emb[sel]
            krel[:nb_, b] = (l[sel] - lo).astype(np.float32)

        # negative pairs: q-th pair of this core at [q % 128, q // 128]
        sl = slice(core * SP, (core + 1) * SP)
        p1 = np.zeros(NPT * 128, np.int64)
        p2 = np.zeros(NPT * 128, np.int64)
        p1[:SP] = i1[sl]
        p2[:SP] = i2[sl]
        nr1 = np.ascontiguousarray(
            emb[p1].reshape(NPT, 128, D).transpose(1, 0, 2)
        )
        nr2 = np.ascontiguousarray(
            emb[p2].reshape(NPT, 128, D).transpose(1, 0, 2)
        )
        meta = np.zeros((128, 32), dtype=np.float32)
        for f, arr in enumerate((l[p1], l[p2], g[p1], g[p2], c[p1], c[p2])):
            meta[:, f * NPT : (f + 1) * NPT] = arr.reshape(NPT, 128).T
        # pad pairs (q >= SP) share row 0 on both sides -> same graph -> masked,
        # but force-invalidate via equal labels in case row 0's graph differs
        padmask = np.zeros(NPT * 128, bool)
        padmask[SP:] = True
        pm2 = padmask.reshape(NPT, 128).T
        meta[:, 0:NPT][pm2] = 0.0
        meta[:, NPT : 2 * NPT][pm2] = 0.0

        in_maps.append(
            {
                "own": own,
                "krel": krel,
                "nr1": nr1,
                "nr2": nr2,
                "meta": meta,
            }
        )
    return in_maps


def kernel(embeddings, labels, graph_ids, categories, idx1, idx2):
    nc = build_program()
    in_maps = make_in_maps(embeddings, labels, graph_ids, categories, idx1, idx2)
    res = run_bass_kernel_spmd(nc, in_maps, list(range(M)))
    out = np.asarray(res.results[0]["out"], dtype=np.float32)
    return out.reshape(())
